# revision 5
# baseline (speedup 1.0000x reference)
"""GQA attention forward on 8 TRN2 NeuronCores, tensor-parallel across heads.

Problem (hardcoded): B=2, T=2048, D=2048, 16 q-heads, 4 kv-heads, head_dim=128,
RoPE (rotate-half pairing i <-> i+64), causal softmax, output projection.

Sharding (per core c of 8):
  q-heads 2c, 2c+1 (rows 256c:256c+256 of wq), kv-head c//2 (rows of wk/wv),
  wo input-dim slice [:, 256c:256c+256]. x replicated. Each core computes a
  full-shape partial of the output (y_local @ wo_slice.T); host sums partials.

v4 design notes (on top of v3):
  - HAM warm-up: ~24 dummy N=512 matmuls on a zeroed tile fill the startup
    DMA window so the PE clock-gate (K=4/8 cold) is released before the
    first real matmul (baseline paid ~8us of throttle_active).
  - All DMA on ONE queue (sync), FIFO-ordered critical-first: wqkv_h0, xt0,
    cb, cs, wqkv_h1, xt1, wo, xt2, xt3. One queue stripes over all 16 DMA
    engines, so priority ordering costs no bandwidth (two queues round-robin
    and delay the critical 4MB).
  - Block-0 projection runs [q kt0-7][kv kt0-7][q kt8-15][kv kt8-15] with
    k/v accumulating in borrowed py banks, giving 6.8us of PE runway on
    half-0 weights while half-1/xt1 stream in.
  - RoPE swap via cross-partition DVE muls (t2[0:64] = raw[64:128]*ssin) --
    drops the 24 PE perm-matmuls. cos/sin table in fp16.
  - Softmax denominator: ONE fp16 accumulation chain + ONE all-ones matmul
    (fused partition-reduce+broadcast). Chain add (~270ns) keeps pace with
    exp production (~720ns/tile).
  - Out-proj steps spread EVENLY across the next block's attention slots
    (2 heads x njt). Late blocks are exp-bound at ~720ns/slot; clustering
    steps in h0 left h1 slots idle at 426ns of PE work.
  - Final bare flush rotates po over 6 banks (po2 + borrowed pj2 + pst2),
    copies alternating DVE/ACT.
  - PSUM = 8 banks: pj(2: q0/q1 then k/v ring), pst(2: score tiles + denom),
    py(2: ps_y, + block-0 k/v borrow), po(2: outproj jb ping-pong).
"""
import math
import numpy as np

P = 128
B = 2
T = 2048
D = 2048
BT = B * T            # 4096
HD = 128              # head dim
QH = 2                # local q heads per core
KT = D // P           # 16 contraction tiles over D
NB = 512              # free-dim block (tokens)
IB = T // NB          # 4 i-blocks per batch
NJT_MAX = T // P      # 16 j-tiles per batch
NCORES = 8
SCALE = 1.0 / math.sqrt(HD)
N_WARM = 24           # HAM warm-up matmuls

_CACHE = {}


def _build():
    import concourse.bass as bass
    import concourse.mybir as mybir
    from concourse import bacc
    from concourse.tile import TileContext

    F32 = mybir.dt.float32
    F16 = mybir.dt.float16
    EXP = mybir.ActivationFunctionType.Exp

    nc = bacc.Bacc("TRN2", target_bir_lowering=False, debug=False)

    # all inputs partition-major-packed on host: [128, ...] contiguous rows
    x_d = nc.dram_tensor("xp", [P, 2 * IB * KT * NB], F16, kind="ExternalInput").ap()
    wqkv_d = nc.dram_tensor("wqkv", [P, KT * 4 * HD], F16, kind="ExternalInput").ap()
    wo_d = nc.dram_tensor("wop", [P, QH * D], F16, kind="ExternalInput").ap()
    cb_d = nc.dram_tensor("cb", [P, 3 * P], F16, kind="ExternalInput").ap()
    cf_d = nc.dram_tensor("cf", [P, 2 * T], F16, kind="ExternalInput").ap()
    out_d = nc.dram_tensor("out", [BT, D], F16, kind="ExternalOutput").ap()

    NHB = 2 * IB * 2   # 16 half-blocks of 8 kt-tiles each
    x_r = x_d.rearrange("p (hb kt m) -> p hb kt m", hb=NHB, kt=KT // 2)
    wqkv_r = wqkv_d.rearrange("p (h kt m) -> p h kt m", h=2, kt=KT // 2)
    wo_r = wo_d.rearrange("p (h j) -> p h j", h=QH)
    cb_r = cb_d.rearrange("p (a q) -> p a q", a=3)
    cf_r = cf_d.rearrange("p (a t) -> p a t", a=2)

    with TileContext(nc) as tc:
        with (
            tc.tile_pool(name="consts", bufs=1) as consts,
            tc.tile_pool(name="acts", bufs=1) as acts,
            tc.tile_pool(name="xt", bufs=4) as xt_pool,
            tc.tile_pool(name="qr", bufs=2) as qr_pool,
            tc.tile_pool(name="raw", bufs=3) as raw_pool,
            tc.tile_pool(name="tt", bufs=2) as t_pool,
            tc.tile_pool(name="est", bufs=6) as est_pool,
            tc.tile_pool(name="accp", bufs=2) as acc_pool,
            tc.tile_pool(name="rinv", bufs=2) as rinv_pool,
            tc.tile_pool(name="ysb", bufs=2) as y_pool,
            tc.tile_pool(name="osb", bufs=2) as o_pool,
            tc.tile_pool(name="pj", bufs=2, space="PSUM") as pj,
            tc.tile_pool(name="pst", bufs=2, space="PSUM") as pst,
            tc.tile_pool(name="py", bufs=2, space="PSUM") as py,
            tc.tile_pool(name="po", bufs=1, space="PSUM") as po,
        ):
            # ---- HAM warm-up: dummy matmuls on zeros fill the DMA window ----
            warm = consts.tile([P, NB], F16)
            nc.vector.memset(warm, 0.0)
            warm_ps = po.tile([P, 2, NB], F32, tag="po", name="warm_ps")
            for _ in range(N_WARM):
                nc.tensor.matmul(warm_ps[:, 0, :], warm[:, 0:P], warm,
                                 start=True, stop=True)

            # ---- resident constants / weights ----
            cb_sb = consts.tile([P, 3, P], F16)
            wqkv_sb = consts.tile([P, 2, KT // 2, 4 * HD], F16)
            cs_sb = consts.tile([P, 2, T], F16)
            wo_sb = consts.tile([P, QH, D], F16)

            # ---- resident activations (per-batch slots) ----
            kr_sb = acts.tile([P, B, T], F16)
            vt_sb = acts.tile([P, B, NJT_MAX, HD], F16)

            xt_tiles = {}

            def prefetch(hb):
                if hb >= NHB or hb in xt_tiles:
                    return
                xt = xt_pool.tile([P, KT // 2, NB], F16, tag="xt", name="xt")
                nc.sync.dma_start(xt, x_r[:, hb])
                xt_tiles[hb] = xt

            # single-queue FIFO, critical-first
            nc.sync.dma_start(wqkv_sb[:, 0], wqkv_r[:, 0])
            prefetch(0)
            nc.sync.dma_start(cb_sb, cb_r)
            nc.sync.dma_start(cs_sb, cf_r)
            nc.sync.dma_start(wqkv_sb[:, 1], wqkv_r[:, 1])
            prefetch(1)
            nc.sync.dma_start(wo_sb, wo_r)
            prefetch(2)
            prefetch(3)

            triu = cb_sb[:, 0, :]
            ident = cb_sb[:, 1, :]
            aones = cb_sb[:, 2, :]
            cos_t = cs_sb[:, 0, :]
            ssin_t = cs_sb[:, 1, :]   # rows 0:64 = -sin, 64:128 = +sin

            def wslc(kt, c0, c1):
                return wqkv_sb[:, kt // 8, kt % 8, c0:c1]

            def rope(ps_raw, dst, t0):
                # dst(fp16) = raw*cos + swap(raw)*ssin; swap via DVE
                # cross-partition muls (no PE involvement)
                raw = raw_pool.tile([P, NB], F16, tag="raw")
                nc.scalar.copy(raw, ps_raw)  # frees the psum bank quickly
                t1 = t_pool.tile([P, NB], F16, tag="t1")
                nc.gpsimd.tensor_mul(t1, raw, cos_t[:, t0:t0 + NB])
                t2 = t_pool.tile([P, NB], F16, tag="t2")
                # ssin is packed input-partition-aligned: rows 64:128 hold
                # -sin (multiplies raw[64:128], lands in t2[0:64]) and rows
                # 0:64 hold +sin -- walrus requires equal base partition for
                # the two SBUF inputs; only the output may be offset.
                nc.vector.tensor_mul(t2[0:64], raw[64:128],
                                     ssin_t[64:128, t0:t0 + NB])
                nc.vector.tensor_mul(t2[64:128], raw[0:64],
                                     ssin_t[0:64, t0:t0 + NB])
                nc.vector.tensor_add(dst, t1, t2)

            def make_outproj_steps(i0p, y_prev, po_tiles):
                steps = []
                state = {}

                def step(s, jb):
                    def run(on_act):
                        bank = po_tiles[(s * (D // NB) + jb) % len(po_tiles)]
                        if jb == 0:
                            state["o"] = o_pool.tile([P, D], F16, tag="o",
                                                     name="o_sb")
                        o_sb = state["o"]
                        nc.tensor.matmul(
                            bank,
                            y_prev[:, 0, s * P:(s + 1) * P],
                            wo_sb[:, 0, jb * NB:(jb + 1) * NB],
                            start=True, stop=False,
                        )
                        nc.tensor.matmul(
                            bank,
                            y_prev[:, 1, s * P:(s + 1) * P],
                            wo_sb[:, 1, jb * NB:(jb + 1) * NB],
                            start=False, stop=True,
                        )
                        dst = o_sb[:, jb * NB:(jb + 1) * NB]
                        if on_act:
                            nc.scalar.copy(dst, bank)
                        else:
                            nc.vector.tensor_copy(dst, bank)
                        if jb == D // NB - 1:
                            row0 = i0p + s * P
                            nc.sync.dma_start(out_d[row0:row0 + P, :], o_sb)
                    return run

                for s in range(NB // P):
                    for jb in range(D // NB):
                        steps.append(step(s, jb))
                return steps

            def emit_proj(b, ib, gblk):
                xta = xt_tiles.pop(2 * gblk)
                xtb = xt_tiles.pop(2 * gblk + 1)
                prefetch(2 * gblk + 4)
                prefetch(2 * gblk + 5)
                t0 = ib * NB

                def xthalf(kt):
                    return (xta if kt < 8 else xtb)[:, kt % 8, :]

                ps_q0 = pj.tile([P, NB], F32, tag="pj", name="ps_q0")
                ps_q1 = pj.tile([P, NB], F32, tag="pj", name="ps_q1")
                if gblk == 0:
                    # startup interleave: q kt0-7, k/v kt0-7 (borrowed py
                    # banks), q kt8-15, k/v kt8-15 -- 6.8us of PE runway on
                    # half-0 data while wqkv_h1/xt1 stream in
                    ps_k = py.tile([P, NB], F32, tag="py", name="ps_k")
                    ps_v = py.tile([P, NB], F32, tag="py", name="ps_v")
                    for half in range(2):
                        for kt in range(8 * half, 8 * half + 8):
                            st, sp = kt == 0, kt == KT - 1
                            nc.tensor.matmul(ps_q0, wslc(kt, 0, P),
                                             xthalf(kt), start=st, stop=sp)
                            nc.tensor.matmul(ps_q1, wslc(kt, P, 2 * P),
                                             xthalf(kt), start=st, stop=sp)
                        for kt in range(8 * half, 8 * half + 8):
                            st, sp = kt == 0, kt == KT - 1
                            nc.tensor.matmul(ps_k, wslc(kt, 2 * P, 3 * P),
                                             xthalf(kt), start=st, stop=sp)
                            nc.tensor.matmul(ps_v, wslc(kt, 3 * P, 4 * P),
                                             xthalf(kt), start=st, stop=sp)
                    qr = qr_pool.tile([P, QH, NB], F16, tag="qr", name="qr")
                    rope(ps_q0, qr[:, 0, :], t0)
                    rope(ps_q1, qr[:, 1, :], t0)
                else:
                    # pass A: the two local q heads
                    for kt in range(KT):
                        st, sp = kt == 0, kt == KT - 1
                        nc.tensor.matmul(ps_q0, wslc(kt, 0, P), xthalf(kt),
                                         start=st, stop=sp)
                        nc.tensor.matmul(ps_q1, wslc(kt, P, 2 * P), xthalf(kt),
                                         start=st, stop=sp)
                    qr = qr_pool.tile([P, QH, NB], F16, tag="qr", name="qr")
                    rope(ps_q0, qr[:, 0, :], t0)
                    rope(ps_q1, qr[:, 1, :], t0)
                    # pass B: k and v for the local kv head (ropes of q
                    # overlap these matmuls)
                    ps_k = pj.tile([P, NB], F32, tag="pj", name="ps_k")
                    ps_v = pj.tile([P, NB], F32, tag="pj", name="ps_v")
                    for kt in range(KT):
                        st, sp = kt == 0, kt == KT - 1
                        nc.tensor.matmul(ps_k, wslc(kt, 2 * P, 3 * P),
                                         xthalf(kt), start=st, stop=sp)
                        nc.tensor.matmul(ps_v, wslc(kt, 3 * P, 4 * P),
                                         xthalf(kt), start=st, stop=sp)
                rope(ps_k, kr_sb[:, b, ib * NB:(ib + 1) * NB], t0)
                vraw = raw_pool.tile([P, NB], F16, tag="raw", name="vraw")
                nc.scalar.copy(vraw, ps_v)
                ps_tr = pj.tile([P, 4, P], F16, tag="pj", name="ps_tr")
                for s4 in range(4):
                    nc.tensor.transpose(ps_tr[:, s4, :],
                                        vraw[:, s4 * P:(s4 + 1) * P], ident)
                nc.vector.tensor_copy(vt_sb[:, b, ib * 4:(ib + 1) * 4, :], ps_tr)
                return qr

            def emit_attn(b, ib, qr, steps):
                y_sb = y_pool.tile([P, QH, NB], F16, tag="y", name="y_sb")
                njt = 4 * ib + 4
                n_slots = QH * njt
                n_steps = len(steps)
                r2 = 2 * n_steps >= n_slots  # >= 1 step/slot on average
                slot = 0
                copy_i = 0

                def fill(slot_i):
                    # spread steps evenly over attention slots
                    nonlocal copy_i
                    lo = n_steps * slot_i // n_slots
                    hi = n_steps * (slot_i + 1) // n_slots
                    for _ in range(hi - lo):
                        mod = 4 if r2 else 8
                        steps.pop(0)(on_act=copy_i % mod == 1)
                        copy_i += 1

                for h in range(QH):
                    ps_y = py.tile([P, NB], F32, tag="py", name="ps_y")
                    acc = acc_pool.tile([P, NB], F16, tag="acc", name="acc")

                    def consume(jt, a, sub, est, slot_i):
                        # mask + denominator-chain add + PV for tile jt,
                        # emitted one tile late so PV's exp dependency has a
                        # full tile-time of slack (PE queue is in-order; an
                        # exp-stalled PV would block the next score matmul)
                        if a >= 0:  # diagonal tile: causal triangle mask
                            nc.vector.tensor_mul(est[:, sub:sub + P],
                                                 est[:, sub:sub + P], triu)
                        if jt == 0:
                            nc.vector.tensor_copy(acc, est)
                        else:
                            nc.vector.tensor_add(acc[:, sub:], acc[:, sub:],
                                                 est[:, sub:])
                        nc.tensor.matmul(
                            ps_y[:, sub:],
                            vt_sb[:, b, jt, :],
                            est[:, sub:],
                            start=jt == 0, stop=jt == njt - 1,
                        )
                        fill(slot_i)

                    pend = None
                    for jt in range(njt):
                        a = jt - 4 * ib
                        sub = max(0, a) * P
                        ps = pst.tile([P, NB], F32, tag="st", name="ps_st")
                        nc.tensor.matmul(
                            ps[:, sub:],
                            kr_sb[:, b, jt * P:(jt + 1) * P],
                            qr[:, h, sub:],
                            start=True, stop=True,
                        )
                        est = est_pool.tile([P, NB], F16, tag="est", name="est")
                        nc.scalar.activation(est[:, sub:], ps[:, sub:], EXP,
                                             scale=SCALE)
                        if pend is not None:
                            consume(*pend)
                        pend = (jt, a, sub, est, slot)
                        slot += 1
                    consume(*pend)
                    # fused partition-reduce + broadcast: every row of the
                    # all-ones matmul output is the per-column denominator
                    rb_ps = pst.tile([P, NB], F32, tag="st", name="rb_ps")
                    nc.tensor.matmul(rb_ps, aones, acc, start=True, stop=True)
                    rinv = rinv_pool.tile([P, NB], F32, tag="rinv", name="rinv")
                    nc.vector.reciprocal_approx_fast(rinv, rb_ps)
                    nc.vector.tensor_mul(y_sb[:, h, :], ps_y, rinv)
                return y_sb

            steps = []
            for b in range(B):
                for ib in range(IB):
                    gblk = b * IB + ib
                    qr = emit_proj(b, ib, gblk)
                    y_sb = emit_attn(b, ib, qr, steps)
                    for i, f in enumerate(steps):  # leftovers (rounding)
                        f(on_act=i % 2 == 1)
                    po_t = po.tile([P, 2, NB], F32, tag="po", name="po_t")
                    po_tiles = [po_t[:, 0, :], po_t[:, 1, :]]
                    if (b, ib) == (B - 1, IB - 1):
                        # last block: projections + attention are done;
                        # borrow pj and pst so the bare final out-proj
                        # rotates over 6 banks
                        po_tiles.append(pj.tile([P, NB], F32, tag="pj",
                                                name="po_t2"))
                        po_tiles.append(pj.tile([P, NB], F32, tag="pj",
                                                name="po_t3"))
                        po_tiles.append(pst.tile([P, NB], F32, tag="st",
                                                 name="po_t4"))
                        po_tiles.append(pst.tile([P, NB], F32, tag="st",
                                                 name="po_t5"))
                    steps = make_outproj_steps(b * T + ib * NB, y_sb, po_tiles)
            for i, f in enumerate(steps):
                f(on_act=i % 2 == 1)

    nc.compile()
    return nc


def _host_prep(x, rope, wq, wk, wv, wo):
    """Build the 8 per-core input maps: shard, fp16, partition-major pack."""
    f16 = np.float16
    xT = x.reshape(BT, D).T.astype(f16)                 # [D, BT]
    xp = np.ascontiguousarray(
        xT.reshape(KT, P, 2 * IB, NB).transpose(1, 2, 0, 3).reshape(P, -1))
    cos = np.asarray(rope[..., 0], dtype=np.float32)    # [T, 64]
    sin = np.asarray(rope[..., 1], dtype=np.float32)
    cosT = np.concatenate([cos.T, cos.T], axis=0)       # [128, T]
    # input-partition-aligned: rows 0:64 = +sin (consumed by raw[0:64]),
    # rows 64:128 = -sin (consumed by raw[64:128])
    ssinT = np.concatenate([sin.T, -sin.T], axis=0)
    cf = np.ascontiguousarray(
        np.concatenate([cosT, ssinT], axis=1).astype(f16))
    triu = np.triu(np.ones((P, P), dtype=np.float32))
    ident = np.eye(P, dtype=np.float32)
    aones = np.ones((P, P), dtype=np.float32)
    cb = np.ascontiguousarray(
        np.concatenate([triu, ident, aones], axis=1).astype(f16))

    in_maps = []
    for c in range(NCORES):
        kv = c // 2
        wqkv = np.concatenate(
            [wq[QH * HD * c:QH * HD * (c + 1), :].T,
             wk[HD * kv:HD * (kv + 1), :].T,
             wv[HD * kv:HD * (kv + 1), :].T], axis=1).astype(f16)  # [D, 512]
        wqkv_p = np.ascontiguousarray(
            wqkv.reshape(KT, P, 4 * HD).transpose(1, 0, 2).reshape(P, -1))
        woT = wo[:, QH * HD * c:QH * HD * (c + 1)].T.astype(f16)   # [256, D]
        wo_p = np.ascontiguousarray(
            woT.reshape(QH, P, D).transpose(1, 0, 2).reshape(P, -1))
        in_maps.append(
            {"xp": xp, "wqkv": wqkv_p, "wop": wo_p, "cb": cb, "cf": cf}
        )
    return in_maps


LAST_RESULTS = None


def kernel(x, rope, wq, wk, wv, wo):
    global LAST_RESULTS
    from concourse import bass_utils

    if "nc" not in _CACHE:
        _CACHE["nc"] = _build()
    nc = _CACHE["nc"]

    in_maps = _host_prep(
        np.asarray(x), np.asarray(rope), np.asarray(wq), np.asarray(wk),
        np.asarray(wv), np.asarray(wo)
    )
    res = bass_utils.run_bass_kernel_spmd(nc, in_maps, core_ids=list(range(NCORES)))
    LAST_RESULTS = res
    acc = np.zeros((BT, D), dtype=np.float64)
    for c in range(NCORES):
        acc += res.results[c]["out"].astype(np.float64)
    return acc.reshape(B, T, D).astype(np.float32)
